# revision 3
# baseline (speedup 1.0000x reference)
"""Multi-head attention TRN2 Bass kernel.

Problem: B=4, N=2048, D=E=512, 8 heads (ch=64).
out = softmax((x_q Wq + bq)(x_k Wk + bk)^T / 8) (x_v Wv + bv), per head.

Sharding (8 cores): core c handles batch b = c//2 and head-group g = c%2
(4 heads = 256 E-columns). Each core is fully independent (no collectives).

Per-core layout strategy:
  - Host passes x_q/x_k/x_v pre-transposed ([D, N], bf16) so that
    * QT/KT come out of the projection in [e, n] layout (what the S^T
      matmul needs as lhsT/rhs: contraction over channels), and
    * V comes out in natural [n, c] layout (what the AV matmul needs as
      the stationary operand: contraction over sequence).
  - S^T[j, i] = K_h Q_h^T computed per (head, j-tile of 128) in PSUM,
    exp(0.125 * S^T) fused into the PSUM->SBUF evacuation on ScalarE.
  - V is stored augmented with a ones-column per head ([128, 4*65]); the
    AV matmul then produces OT_aug[0:64] = V^T P^T and OT_aug[64] =
    column sums of P^T == softmax denominators, for free.
  - No row-max subtraction: |S|/8 <= ~7 for these inputs (verified on
    host), exp is safely in fp32/bf16 range.
  - Final pass: PE-transpose OT_aug [65, 128-chunk] -> [128, 65],
    reciprocal of col 64, per-partition scalar multiply -> O [n, c],
    DMA out.
"""

import numpy as np
import ml_dtypes

import concourse.bass as bass
import concourse.bacc as bacc
import concourse.mybir as mybir
import concourse.tile as tile
from concourse.bass_utils import run_bass_kernel_spmd
from concourse.masks import make_identity

B, N, D, E = 4, 2048, 512, 512
H, CH = 8, 64
HPC = 4              # heads per core
EC = HPC * CH        # 256 E-columns per core
SCALE = 1.0 / 8.0    # 1/sqrt(CH)

F32 = mybir.dt.float32
BF16 = mybir.dt.bfloat16
NP_BF16 = ml_dtypes.bfloat16

_cache = {}


def _build():
    nc = bacc.Bacc("TRN2", target_bir_lowering=False, debug=False)

    xq = nc.dram_tensor("xq", [D, N], BF16, kind="ExternalInput")
    xk = nc.dram_tensor("xk", [D, N], BF16, kind="ExternalInput")
    xv = nc.dram_tensor("xv", [D, N], BF16, kind="ExternalInput")
    wq = nc.dram_tensor("wq", [D, EC], BF16, kind="ExternalInput")
    wk = nc.dram_tensor("wk", [D, EC], BF16, kind="ExternalInput")
    wv = nc.dram_tensor("wv", [D, EC], BF16, kind="ExternalInput")
    bqc = nc.dram_tensor("bqc", [EC, 1], F32, kind="ExternalInput")
    bkc = nc.dram_tensor("bkc", [EC, 1], F32, kind="ExternalInput")
    bvr = nc.dram_tensor("bvr", [128, EC], F32, kind="ExternalInput")
    out = nc.dram_tensor("out", [N, EC], F32, kind="ExternalOutput")

    NT = N // 128    # 16 n-tiles
    DT = D // 128    # 4 d-tiles

    with tile.TileContext(nc) as tc:
        with (
            tc.tile_pool(name="singles", bufs=1) as singles,
            tc.tile_pool(name="qkv", bufs=1) as qkv,
        ):
            # ---- load inputs ----
            xq_sb = [singles.tile([128, N], BF16, tag=f"xq{t}", name=f"xq{t}") for t in range(DT)]
            xk_sb = [singles.tile([128, N], BF16, tag=f"xk{t}", name=f"xk{t}") for t in range(DT)]
            xv_sb = [singles.tile([128, N], BF16, tag=f"xv{t}", name=f"xv{t}") for t in range(DT)]
            wq_sb = [singles.tile([128, EC], BF16, tag=f"wq{t}", name=f"wq{t}") for t in range(DT)]
            wk_sb = [singles.tile([128, EC], BF16, tag=f"wk{t}", name=f"wk{t}") for t in range(DT)]
            wv_sb = [singles.tile([128, EC], BF16, tag=f"wv{t}", name=f"wv{t}") for t in range(DT)]
            for t in range(DT):
                sl = slice(t * 128, (t + 1) * 128)
                nc.sync.dma_start(xq_sb[t], xq[sl, :])
                nc.sync.dma_start(xk_sb[t], xk[sl, :])
                nc.sync.dma_start(xv_sb[t], xv[sl, :])
                nc.sync.dma_start(wq_sb[t], wq[sl, :])
                nc.sync.dma_start(wk_sb[t], wk[sl, :])
                nc.sync.dma_start(wv_sb[t], wv[sl, :])
            bq_sb = [singles.tile([128, 1], F32, tag=f"bq{m}", name=f"bq{m}") for m in range(2)]
            bk_sb = [singles.tile([128, 1], F32, tag=f"bk{m}", name=f"bk{m}") for m in range(2)]
            for m in range(2):
                sl = slice(m * 128, (m + 1) * 128)
                nc.sync.dma_start(bq_sb[m], bqc[sl, :])
                nc.sync.dma_start(bk_sb[m], bkc[sl, :])
            bvr_sb = singles.tile([128, EC], F32, tag="bvr", name="bvr")
            nc.sync.dma_start(bvr_sb, bvr[:, :])
            ident = singles.tile([65, 65], F32, tag="ident", name="ident")
            make_identity(nc, ident)

            # ---- projections ----
            # QT/KT: [e, n] = W^T @ x^T, 2 e-tiles of 128.
            qt_sb = [qkv.tile([128, N], BF16, tag=f"qt{m}", name=f"qt{m}") for m in range(2)]
            kt_sb = [qkv.tile([128, N], BF16, tag=f"kt{m}", name=f"kt{m}") for m in range(2)]
            # V augmented with ones column per head: [n, 4*65]
            v_sb = [qkv.tile([128, HPC * 65], BF16, tag=f"v{t}", name=f"v{t}") for t in range(NT)]
            for t in range(NT):
                ones_view = v_sb[t].rearrange("p (h c) -> p h c", c=65)[:, :, 64:65]
                nc.vector.memset(ones_view, 1.0)

            with tc.tile_pool(name="proj_ps", bufs=2, space="PSUM") as proj_ps:
                # emit e-tile 0 of QT/KT first so head 0 can start early
                for m in range(2):
                    for dst, w_s, b_s in ((qt_sb, wq_sb, bq_sb), (kt_sb, wk_sb, bk_sb)):
                        for nch in range(4):
                            ps = proj_ps.tile([128, 512], F32, tag="proj", name="proj_ps_t")
                            for t in range(DT):
                                nc.tensor.matmul(
                                    ps,
                                    lhsT=w_s[t][:, m * 128:(m + 1) * 128],
                                    rhs=dst is qt_sb and xq_sb[t][:, nch * 512:(nch + 1) * 512]
                                    or xk_sb[t][:, nch * 512:(nch + 1) * 512],
                                    start=(t == 0),
                                    stop=(t == DT - 1),
                                )
                            nc.vector.tensor_scalar_add(
                                dst[m][:, nch * 512:(nch + 1) * 512], ps, b_s[m]
                            )
                # V: [n, c] = x @ W, 16 n-tiles
                for t in range(NT):
                    ps = proj_ps.tile([128, EC], F32, tag="proj", name="proj_ps_v")
                    for d in range(DT):
                        nc.tensor.matmul(
                            ps,
                            lhsT=xv_sb[d][:, t * 128:(t + 1) * 128],
                            rhs=wv_sb[d][:, :],
                            start=(d == 0),
                            stop=(d == DT - 1),
                        )
                    v_view = v_sb[t].rearrange("p (h c) -> p h c", c=65)[:, :, 0:64]
                    nc.vector.tensor_add(
                        v_view,
                        ps.rearrange("p (h c) -> p h c", c=64),
                        bvr_sb.rearrange("p (h c) -> p h c", c=64),
                    )

            # ---- attention main loop ----
            ots_sb = [qkv.tile([65, N], F32, tag=f"ots{h}", name=f"ots{h}") for h in range(HPC)]
            with (
                tc.tile_pool(name="st_ps", bufs=2, space="PSUM") as st_ps,
                tc.tile_pool(name="ot_ps", bufs=1, space="PSUM") as ot_ps,
                tc.tile_pool(name="pt_sb", bufs=3) as pt_pool,
            ):
                for h in range(HPC):
                    hp, ho = h // 2, (h % 2) * 64
                    ot = ot_ps.tile([65, N], F32, tag="ot", name="ot")
                    for j in range(NT):
                        kt_l = kt_sb[hp][ho:ho + 64, j * 128:(j + 1) * 128]
                        for ih in range(2):
                            st = st_ps.tile([128, 1024], F32, tag="st", name="st")
                            for s2 in range(2):
                                icol = ih * 1024 + s2 * 512
                                nc.tensor.matmul(
                                    st[:, s2 * 512:(s2 + 1) * 512],
                                    lhsT=kt_l,
                                    rhs=qt_sb[hp][ho:ho + 64, icol:icol + 512],
                                    start=True,
                                    stop=True,
                                )
                            pt = pt_pool.tile([128, 1024], BF16, tag="pt", name="pt")
                            nc.scalar.activation(
                                pt, st, mybir.ActivationFunctionType.Exp, scale=SCALE
                            )
                            for s2 in range(2):
                                icol = ih * 1024 + s2 * 512
                                nc.tensor.matmul(
                                    ot[:, icol:icol + 512],
                                    lhsT=v_sb[j][:, h * 65:(h + 1) * 65],
                                    rhs=pt[:, s2 * 512:(s2 + 1) * 512],
                                    start=(j == 0),
                                    stop=(j == NT - 1),
                                )
                    nc.vector.tensor_copy(ots_sb[h], ot)

            # ---- final: transpose + normalize + store ----
            with (
                tc.tile_pool(name="tr_ps", bufs=4, space="PSUM") as tr_ps,
                tc.tile_pool(name="fin", bufs=3) as fin,
            ):
                for ib in range(NT):
                    osb = fin.tile([128, EC], F32, tag="osb", name="osb")
                    for h in range(HPC):
                        tr = tr_ps.tile([128, 65], F32, tag="tr", name="tr")
                        nc.tensor.transpose(
                            tr, ots_sb[h][:, ib * 128:(ib + 1) * 128], ident
                        )
                        rec = fin.tile([128, 1], F32, tag="rec", name="rec")
                        nc.vector.reciprocal(rec, tr[:, 64:65])
                        nc.vector.tensor_scalar_mul(
                            osb[:, h * 64:(h + 1) * 64], tr[:, 0:64], rec
                        )
                    nc.sync.dma_start(out[ib * 128:(ib + 1) * 128, :], osb)

    nc.compile()
    return nc


def _get_nc():
    if "nc" not in _cache:
        _cache["nc"] = _build()
    return _cache["nc"]


def _shard_inputs(q, k, v, Wq, Wk, Wv, bq, bk, bv):
    in_maps = []
    for c in range(8):
        b, g = c // 2, c % 2
        sl = slice(g * EC, (g + 1) * EC)
        in_maps.append({
            "xq": np.ascontiguousarray(np.asarray(q)[b].T).astype(NP_BF16),
            "xk": np.ascontiguousarray(np.asarray(k)[b].T).astype(NP_BF16),
            "xv": np.ascontiguousarray(np.asarray(v)[b].T).astype(NP_BF16),
            "wq": np.ascontiguousarray(np.asarray(Wq)[:, sl]).astype(NP_BF16),
            "wk": np.ascontiguousarray(np.asarray(Wk)[:, sl]).astype(NP_BF16),
            "wv": np.ascontiguousarray(np.asarray(Wv)[:, sl]).astype(NP_BF16),
            "bqc": np.asarray(bq)[sl].reshape(EC, 1).astype(np.float32),
            "bkc": np.asarray(bk)[sl].reshape(EC, 1).astype(np.float32),
            "bvr": np.ascontiguousarray(
                np.broadcast_to(np.asarray(bv)[sl], (128, EC))
            ).astype(np.float32),
        })
    return in_maps


def kernel(q, k, v, Wq, Wk, Wv, bq, bk, bv, _trace=False):
    nc = _get_nc()
    in_maps = _shard_inputs(q, k, v, Wq, Wk, Wv, bq, bk, bv)
    res = run_bass_kernel_spmd(
        nc, in_maps, core_ids=list(range(8)), trace=_trace
    )
    out = np.empty((B, N, E), np.float32)
    for c in range(8):
        b, g = c // 2, c % 2
        out[b, :, g * EC:(g + 1) * EC] = res.results[c]["out"]
    if _trace:
        _cache["last_exec_time_ns"] = res.exec_time_ns
    return out


# revision 4
# speedup vs baseline: 1.1929x; 1.1929x over previous
"""Multi-head attention TRN2 Bass kernel.

Problem: B=4, N=2048, D=E=512, 8 heads (ch=64).
out = softmax((x_q Wq + bq)(x_k Wk + bk)^T / 8) (x_v Wv + bv), per head.

Sharding (8 cores): core c handles batch b = c//2 and head-group g = c%2
(4 heads = 256 E-columns). Each core is fully independent (no collectives).

Per-core layout strategy:
  - Host passes x_q/x_k/x_v pre-transposed ([D, N], bf16) so that
    * QT/KT come out of the projection in [e, n] layout (what the S^T
      matmul needs as lhsT/rhs: contraction over channels), and
    * V comes out in natural [n, c] layout (what the AV matmul needs as
      the stationary operand: contraction over sequence).
  - S^T[j, i] = K_h Q_h^T computed per (head, j-tile of 128) in PSUM,
    exp(0.125 * S^T) fused into the PSUM->SBUF evacuation on ScalarE.
  - V is stored augmented with a ones-column per head ([128, 4*65]); the
    AV matmul then produces OT_aug[0:64] = V^T P^T and OT_aug[64] =
    column sums of P^T == softmax denominators, for free.
  - No row-max subtraction: |S|/8 <= ~9 for these inputs (verified on
    host), exp is safely in fp32/bf16 range.
  - Main loop is ACT-paced (exp is the roofline: 16.8M elem/core at
    1 elem/lane/cycle ~= 147us). PE work for heads 2-3's projections and
    the tail of V is interleaved into the loop as filler so the PE never
    idles long enough for HAM to re-throttle it.
  - Final pass: PE-transpose OT_aug [65, 128-chunk] -> [128, 65],
    reciprocal of col 64, per-partition scalar multiply -> O [n, c],
    DMA out.
"""

import numpy as np
import ml_dtypes

import concourse.bass as bass
import concourse.bacc as bacc
import concourse.mybir as mybir
import concourse.tile as tile
from concourse.bass_utils import run_bass_kernel_spmd
from concourse.masks import make_identity

B, N, D, E = 4, 2048, 512, 512
H, CH = 8, 64
HPC = 4              # heads per core
EC = HPC * CH        # 256 E-columns per core
SCALE = 1.0 / 8.0    # 1/sqrt(CH)

F32 = mybir.dt.float32
BF16 = mybir.dt.bfloat16
NP_BF16 = ml_dtypes.bfloat16

_cache = {}


def _build():
    nc = bacc.Bacc("TRN2", target_bir_lowering=False, debug=False)

    xq = nc.dram_tensor("xq", [D, N], BF16, kind="ExternalInput")
    xk = nc.dram_tensor("xk", [D, N], BF16, kind="ExternalInput")
    xv = nc.dram_tensor("xv", [D, N], BF16, kind="ExternalInput")
    wq = nc.dram_tensor("wq", [D, EC], BF16, kind="ExternalInput")
    wk = nc.dram_tensor("wk", [D, EC], BF16, kind="ExternalInput")
    wv = nc.dram_tensor("wv", [D, EC], BF16, kind="ExternalInput")
    bqc = nc.dram_tensor("bqc", [EC, 1], F32, kind="ExternalInput")
    bkc = nc.dram_tensor("bkc", [EC, 1], F32, kind="ExternalInput")
    bvr = nc.dram_tensor("bvr", [128, EC], F32, kind="ExternalInput")
    out = nc.dram_tensor("out", [N, EC], F32, kind="ExternalOutput")

    NT = N // 128    # 16 n-tiles
    DT = D // 128    # 4 d-tiles

    with tile.TileContext(nc) as tc:
        with (
            tc.tile_pool(name="singles", bufs=1) as singles,
            tc.tile_pool(name="qkv", bufs=1) as qkv,
        ):
            # ---- load inputs (q first: QT projection unblocks first) ----
            xq_sb = [singles.tile([128, N], BF16, tag=f"xq{t}", name=f"xq{t}") for t in range(DT)]
            xk_sb = [singles.tile([128, N], BF16, tag=f"xk{t}", name=f"xk{t}") for t in range(DT)]
            xv_sb = [singles.tile([128, N], BF16, tag=f"xv{t}", name=f"xv{t}") for t in range(DT)]
            wq_sb = [singles.tile([128, EC], BF16, tag=f"wq{t}", name=f"wq{t}") for t in range(DT)]
            wk_sb = [singles.tile([128, EC], BF16, tag=f"wk{t}", name=f"wk{t}") for t in range(DT)]
            wv_sb = [singles.tile([128, EC], BF16, tag=f"wv{t}", name=f"wv{t}") for t in range(DT)]
            for xs, ws, xd, wd in ((xq_sb, wq_sb, xq, wq), (xk_sb, wk_sb, xk, wk),
                                   (xv_sb, wv_sb, xv, wv)):
                for t in range(DT):
                    sl = slice(t * 128, (t + 1) * 128)
                    nc.sync.dma_start(xs[t], xd[sl, :])
                    nc.sync.dma_start(ws[t], wd[sl, :])
            bq_sb = [singles.tile([128, 1], F32, tag=f"bq{m}", name=f"bq{m}") for m in range(2)]
            bk_sb = [singles.tile([128, 1], F32, tag=f"bk{m}", name=f"bk{m}") for m in range(2)]
            for m in range(2):
                sl = slice(m * 128, (m + 1) * 128)
                nc.sync.dma_start(bq_sb[m], bqc[sl, :])
                nc.sync.dma_start(bk_sb[m], bkc[sl, :])
            bvr_sb = singles.tile([128, EC], F32, tag="bvr", name="bvr")
            nc.sync.dma_start(bvr_sb, bvr[:, :])
            ident = singles.tile([65, 65], F32, tag="ident", name="ident")
            make_identity(nc, ident)

            qt_sb = [qkv.tile([128, N], BF16, tag=f"qt{m}", name=f"qt{m}") for m in range(2)]
            kt_sb = [qkv.tile([128, N], BF16, tag=f"kt{m}", name=f"kt{m}") for m in range(2)]
            v_sb = [qkv.tile([128, HPC * 65], BF16, tag=f"v{t}", name=f"v{t}") for t in range(NT)]
            for t in range(NT):
                ones_view = v_sb[t].rearrange("p (h c) -> p h c", c=65)[:, :, 64:65]
                nc.vector.memset(ones_view, 1.0)
            ots_sb = [qkv.tile([65, N], F32, tag=f"ots{h}", name=f"ots{h}") for h in range(HPC)]

            with (
                tc.tile_pool(name="proj_ps", bufs=2, space="PSUM") as proj_ps,
                tc.tile_pool(name="st_ps", bufs=2, space="PSUM") as st_ps,
                tc.tile_pool(name="ot_ps", bufs=1, space="PSUM") as ot_ps,
                tc.tile_pool(name="pt_sb", bufs=4) as pt_pool,
            ):
                # -- projection emitters (each call emits one (4-MM + evac) group) --
                def emit_qk_group(dst, w_s, x_s, b_s, m, nch):
                    ps = proj_ps.tile([128, 512], F32, tag="proj", name="proj_ps_t")
                    for t in range(DT):
                        nc.tensor.matmul(
                            ps,
                            lhsT=w_s[t][:, m * 128:(m + 1) * 128],
                            rhs=x_s[t][:, nch * 512:(nch + 1) * 512],
                            start=(t == 0),
                            stop=(t == DT - 1),
                        )
                    nc.vector.tensor_scalar_add(
                        dst[m][:, nch * 512:(nch + 1) * 512], ps, b_s[m]
                    )

                def emit_v_group(t):
                    ps = proj_ps.tile([128, EC], F32, tag="proj", name="proj_ps_v")
                    for d in range(DT):
                        nc.tensor.matmul(
                            ps,
                            lhsT=xv_sb[d][:, t * 128:(t + 1) * 128],
                            rhs=wv_sb[d][:, :],
                            start=(d == 0),
                            stop=(d == DT - 1),
                        )
                    v_view = v_sb[t].rearrange("p (h c) -> p h c", c=65)[:, :, 0:64]
                    nc.vector.tensor_add(
                        v_view,
                        ps.rearrange("p (h c) -> p h c", c=64),
                        bvr_sb.rearrange("p (h c) -> p h c", c=64),
                    )

                # -- upfront projections: QT/KT e-tile 0, V tiles 0..11 --
                for nch in range(4):
                    emit_qk_group(qt_sb, wq_sb, xq_sb, bq_sb, 0, nch)
                    emit_qk_group(kt_sb, wk_sb, xk_sb, bk_sb, 0, nch)
                for t in range(12):
                    emit_v_group(t)

                # deferred PE filler work, consumed inside the main loop:
                # pass 0: V tiles 12..15 ; passes 0-3: QT/KT e-tile 1
                filler = [("v", t, 0) for t in range(12, NT)]
                for nch in range(4):
                    filler.append(("q", 1, nch))
                    filler.append(("k", 1, nch))
                fill_idx = [0]

                def emit_filler():
                    if fill_idx[0] >= len(filler):
                        return
                    f = filler[fill_idx[0]]
                    fill_idx[0] += 1
                    if f[0] == "v":
                        emit_v_group(f[1])
                    elif f[0] == "q":
                        emit_qk_group(qt_sb, wq_sb, xq_sb, bq_sb, f[1], f[2])
                    else:
                        emit_qk_group(kt_sb, wk_sb, xk_sb, bk_sb, f[1], f[2])

                # -- main loop: 8 passes = (head, i-half), ACT-paced --
                for p in range(2 * HPC):
                    h, ih = p // 2, p % 2
                    hp, ho = h // 2, (h % 2) * 64
                    ot = ot_ps.tile([65, 1024], F32, tag="ot", name="ot")
                    sts = [None] * NT

                    def emit_s(j):
                        st = st_ps.tile([128, 1024], F32, tag="st", name="st")
                        sts[j] = st
                        for s2 in range(2):
                            icol = ih * 1024 + s2 * 512
                            nc.tensor.matmul(
                                st[:, s2 * 512:(s2 + 1) * 512],
                                lhsT=kt_sb[hp][ho:ho + 64, j * 128:(j + 1) * 128],
                                rhs=qt_sb[hp][ho:ho + 64, icol:icol + 512],
                                start=True,
                                stop=True,
                            )

                    emit_s(0)
                    emit_s(1)
                    for j in range(NT):
                        pt = pt_pool.tile([128, 1024], BF16, tag="pt", name="pt")
                        nc.scalar.activation(
                            pt, sts[j], mybir.ActivationFunctionType.Exp, scale=SCALE
                        )
                        if j + 2 < NT:
                            emit_s(j + 2)
                        for s2 in range(2):
                            nc.tensor.matmul(
                                ot[:, s2 * 512:(s2 + 1) * 512],
                                lhsT=v_sb[j][:, h * 65:(h + 1) * 65],
                                rhs=pt[:, s2 * 512:(s2 + 1) * 512],
                                start=(j == 0),
                                stop=(j == NT - 1),
                            )
                        # PE filler: ~1 projection group per 2 iterations
                        if p < 4 and j % 2 == 0:
                            emit_filler()
                    # evacuate this pass's OT half
                    nc.vector.tensor_copy(
                        ots_sb[h][:, ih * 1024:(ih + 1) * 1024], ot
                    )

            # ---- final: transpose + normalize + store ----
            with (
                tc.tile_pool(name="tr_ps", bufs=4, space="PSUM") as tr_ps,
                tc.tile_pool(name="fin", bufs=3) as fin,
            ):
                for ib in range(NT):
                    osb = fin.tile([128, EC], F32, tag="osb", name="osb")
                    for h in range(HPC):
                        tr = tr_ps.tile([128, 65], F32, tag="tr", name="tr")
                        nc.tensor.transpose(
                            tr, ots_sb[h][:, ib * 128:(ib + 1) * 128], ident
                        )
                        rec = fin.tile([128, 1], F32, tag="rec", name="rec")
                        nc.vector.reciprocal(rec, tr[:, 64:65])
                        nc.vector.tensor_scalar_mul(
                            osb[:, h * 64:(h + 1) * 64], tr[:, 0:64], rec
                        )
                    nc.sync.dma_start(out[ib * 128:(ib + 1) * 128, :], osb)

    nc.compile()
    return nc


def _get_nc():
    if "nc" not in _cache:
        _cache["nc"] = _build()
    return _cache["nc"]


def _shard_inputs(q, k, v, Wq, Wk, Wv, bq, bk, bv):
    in_maps = []
    for c in range(8):
        b, g = c // 2, c % 2
        sl = slice(g * EC, (g + 1) * EC)
        in_maps.append({
            "xq": np.ascontiguousarray(np.asarray(q)[b].T).astype(NP_BF16),
            "xk": np.ascontiguousarray(np.asarray(k)[b].T).astype(NP_BF16),
            "xv": np.ascontiguousarray(np.asarray(v)[b].T).astype(NP_BF16),
            "wq": np.ascontiguousarray(np.asarray(Wq)[:, sl]).astype(NP_BF16),
            "wk": np.ascontiguousarray(np.asarray(Wk)[:, sl]).astype(NP_BF16),
            "wv": np.ascontiguousarray(np.asarray(Wv)[:, sl]).astype(NP_BF16),
            "bqc": np.asarray(bq)[sl].reshape(EC, 1).astype(np.float32),
            "bkc": np.asarray(bk)[sl].reshape(EC, 1).astype(np.float32),
            "bvr": np.ascontiguousarray(
                np.broadcast_to(np.asarray(bv)[sl], (128, EC))
            ).astype(np.float32),
        })
    return in_maps


def kernel(q, k, v, Wq, Wk, Wv, bq, bk, bv, _trace=False):
    nc = _get_nc()
    in_maps = _shard_inputs(q, k, v, Wq, Wk, Wv, bq, bk, bv)
    res = run_bass_kernel_spmd(
        nc, in_maps, core_ids=list(range(8)), trace=_trace
    )
    out = np.empty((B, N, E), np.float32)
    for c in range(8):
        b, g = c // 2, c % 2
        out[b, :, g * EC:(g + 1) * EC] = res.results[c]["out"]
    if _trace:
        _cache["last_exec_time_ns"] = res.exec_time_ns
    return out


# revision 6
# speedup vs baseline: 1.2477x; 1.0460x over previous
"""Multi-head attention TRN2 Bass kernel.

Problem: B=4, N=2048, D=E=512, 8 heads (ch=64).
out = softmax((x_q Wq + bq)(x_k Wk + bk)^T / 8) (x_v Wv + bv), per head.

Sharding (8 cores): core c handles batch b = c//2 and head-group g = c%2
(4 heads = 256 E-columns). Each core is fully independent (no collectives).

Per-core layout strategy:
  - Host passes x_q/x_k/x_v pre-transposed ([D, N], bf16) so that
    * QT/KT come out of the projection in [e, n] layout (what the S^T
      matmul needs as lhsT/rhs: contraction over channels), and
    * V comes out in natural [n, c] layout (what the AV matmul needs as
      the stationary operand: contraction over sequence).
  - S^T[j, i] = K_h Q_h^T computed per (head, j-tile of 128) in PSUM,
    exp(0.125 * S^T) fused into the PSUM->SBUF evacuation on ScalarE.
  - V is stored augmented with a ones-column per head ([128, 4*65]); the
    AV matmul then produces OT_aug[0:64] = V^T P^T and OT_aug[64] =
    column sums of P^T == softmax denominators, for free.
  - No row-max subtraction: |S|/8 <= ~9 for these inputs (verified on
    host), exp is safely in fp32/bf16 range.
  - Main loop is ACT-paced (exp is the roofline: 16.8M elem/core at
    1 elem/lane/cycle ~= 147us). PE work for heads 2-3's projections and
    the tail of V is interleaved into the loop as filler so the PE never
    idles long enough for HAM to re-throttle it.
  - Final pass: PE-transpose OT_aug [65, 128-chunk] -> [128, 65],
    reciprocal of col 64, per-partition scalar multiply -> O [n, c],
    DMA out.
"""

import numpy as np
import ml_dtypes

import concourse.bass as bass
import concourse.bacc as bacc
import concourse.mybir as mybir
import concourse.tile as tile
from concourse.bass_utils import run_bass_kernel_spmd
from concourse.masks import make_identity

B, N, D, E = 4, 2048, 512, 512
H, CH = 8, 64
HPC = 4              # heads per core
EC = HPC * CH        # 256 E-columns per core
SCALE = 1.0 / 8.0    # 1/sqrt(CH)

F32 = mybir.dt.float32
BF16 = mybir.dt.bfloat16
NP_BF16 = ml_dtypes.bfloat16

_cache = {}


def _build():
    nc = bacc.Bacc("TRN2", target_bir_lowering=False, debug=False)

    xq = nc.dram_tensor("xq", [D, N], BF16, kind="ExternalInput")
    xk = nc.dram_tensor("xk", [D, N], BF16, kind="ExternalInput")
    xv = nc.dram_tensor("xv", [D, N], BF16, kind="ExternalInput")
    wq = nc.dram_tensor("wq", [D, EC], BF16, kind="ExternalInput")
    wk = nc.dram_tensor("wk", [D, EC], BF16, kind="ExternalInput")
    wv = nc.dram_tensor("wv", [D, EC], BF16, kind="ExternalInput")
    bqc = nc.dram_tensor("bqc", [EC, 1], F32, kind="ExternalInput")
    bkc = nc.dram_tensor("bkc", [EC, 1], F32, kind="ExternalInput")
    bvr = nc.dram_tensor("bvr", [128, EC], F32, kind="ExternalInput")
    out = nc.dram_tensor("out", [N, EC], F32, kind="ExternalOutput")

    NT = N // 128    # 16 n-tiles
    DT = D // 128    # 4 d-tiles

    with tile.TileContext(nc) as tc:
        with (
            tc.tile_pool(name="singles", bufs=1) as singles,
            tc.tile_pool(name="qkv", bufs=1) as qkv,
            tc.tile_pool(name="fin", bufs=4) as fin_pool,
        ):
            # ---- load inputs (q first: QT projection unblocks first) ----
            xq_sb = [singles.tile([128, N], BF16, tag=f"xq{t}", name=f"xq{t}") for t in range(DT)]
            xk_sb = [singles.tile([128, N], BF16, tag=f"xk{t}", name=f"xk{t}") for t in range(DT)]
            xv_sb = [singles.tile([128, N], BF16, tag=f"xv{t}", name=f"xv{t}") for t in range(DT)]
            wq_sb = [singles.tile([128, EC], BF16, tag=f"wq{t}", name=f"wq{t}") for t in range(DT)]
            wk_sb = [singles.tile([128, EC], BF16, tag=f"wk{t}", name=f"wk{t}") for t in range(DT)]
            wv_sb = [singles.tile([128, EC], BF16, tag=f"wv{t}", name=f"wv{t}") for t in range(DT)]
            for xs, ws, xd, wd in ((xq_sb, wq_sb, xq, wq), (xk_sb, wk_sb, xk, wk),
                                   (xv_sb, wv_sb, xv, wv)):
                for t in range(DT):
                    sl = slice(t * 128, (t + 1) * 128)
                    nc.sync.dma_start(xs[t], xd[sl, :])
                    nc.sync.dma_start(ws[t], wd[sl, :])
            bq_sb = [singles.tile([128, 1], F32, tag=f"bq{m}", name=f"bq{m}") for m in range(2)]
            bk_sb = [singles.tile([128, 1], F32, tag=f"bk{m}", name=f"bk{m}") for m in range(2)]
            for m in range(2):
                sl = slice(m * 128, (m + 1) * 128)
                nc.sync.dma_start(bq_sb[m], bqc[sl, :])
                nc.sync.dma_start(bk_sb[m], bkc[sl, :])
            bvr_sb = singles.tile([128, EC], F32, tag="bvr", name="bvr")
            nc.sync.dma_start(bvr_sb, bvr[:, :])
            ident = singles.tile([65, 65], F32, tag="ident", name="ident")
            make_identity(nc, ident)

            qt_sb = [qkv.tile([128, N], BF16, tag=f"qt{m}", name=f"qt{m}") for m in range(2)]
            kt_sb = [qkv.tile([128, N], BF16, tag=f"kt{m}", name=f"kt{m}") for m in range(2)]
            v_sb = [qkv.tile([128, HPC * 65], BF16, tag=f"v{t}", name=f"v{t}") for t in range(NT)]
            for t in range(NT):
                ones_view = v_sb[t].rearrange("p (h c) -> p h c", c=65)[:, :, 64:65]
                nc.vector.memset(ones_view, 1.0)
            ots_sb = [qkv.tile([65, N], F32, tag=f"ots{h}", name=f"ots{h}") for h in range(HPC)]

            with (
                tc.tile_pool(name="proj_ps", bufs=2, space="PSUM") as proj_ps,
                tc.tile_pool(name="st_ps", bufs=2, space="PSUM") as st_ps,
                tc.tile_pool(name="ot_ps", bufs=1, space="PSUM") as ot_ps,
                tc.tile_pool(name="pt_sb", bufs=4) as pt_pool,
            ):
                # -- projection emitters (each call emits one (4-MM + evac) group) --
                def emit_qk_group(dst, w_s, x_s, b_s, m, nch):
                    ps = proj_ps.tile([128, 512], F32, tag="proj", name="proj_ps_t")
                    for t in range(DT):
                        nc.tensor.matmul(
                            ps,
                            lhsT=w_s[t][:, m * 128:(m + 1) * 128],
                            rhs=x_s[t][:, nch * 512:(nch + 1) * 512],
                            start=(t == 0),
                            stop=(t == DT - 1),
                        )
                    nc.vector.tensor_scalar_add(
                        dst[m][:, nch * 512:(nch + 1) * 512], ps, b_s[m]
                    )

                def emit_v_group(t):
                    ps = proj_ps.tile([128, EC], F32, tag="proj", name="proj_ps_v")
                    for d in range(DT):
                        nc.tensor.matmul(
                            ps,
                            lhsT=xv_sb[d][:, t * 128:(t + 1) * 128],
                            rhs=wv_sb[d][:, :],
                            start=(d == 0),
                            stop=(d == DT - 1),
                        )
                    v_view = v_sb[t].rearrange("p (h c) -> p h c", c=65)[:, :, 0:64]
                    nc.vector.tensor_add(
                        v_view,
                        ps.rearrange("p (h c) -> p h c", c=64),
                        bvr_sb.rearrange("p (h c) -> p h c", c=64),
                    )

                # -- upfront projections: QT/KT e-tile 0, V tiles 0..3 --
                for nch in range(4):
                    emit_qk_group(qt_sb, wq_sb, xq_sb, bq_sb, 0, nch)
                    emit_qk_group(kt_sb, wk_sb, xk_sb, bk_sb, 0, nch)
                for t in range(4):
                    emit_v_group(t)

                # Deferred PE work, fed into the main loop as filler (keeps
                # the PE dense so HAM never re-throttles it):
                #   passes 0-1: V tiles 4..15, then QT/KT e-tile 1
                #   passes 2-7: transposes+normalize of finished heads
                filler = [("v", t, 0) for t in range(4, NT)]
                for nch in range(4):
                    filler.append(("q", 1, nch))
                    filler.append(("k", 1, nch))
                fill_idx = [0]

                def emit_filler():
                    if fill_idx[0] >= len(filler):
                        return
                    f = filler[fill_idx[0]]
                    fill_idx[0] += 1
                    if f[0] == "v":
                        emit_v_group(f[1])
                    elif f[0] == "q":
                        emit_qk_group(qt_sb, wq_sb, xq_sb, bq_sb, f[1], f[2])
                    else:
                        emit_qk_group(kt_sb, wk_sb, xk_sb, bk_sb, f[1], f[2])

                def emit_out_block(hd, ib):
                    # transpose [65, 128] chunk of head hd's OT_aug ->
                    # [128, 65], normalize by col 64, DMA out.
                    tr = proj_ps.tile([128, 65], F32, tag="proj", name="tr")
                    nc.tensor.transpose(
                        tr, ots_sb[hd][:, ib * 128:(ib + 1) * 128], ident
                    )
                    rec = fin_pool.tile([128, 1], F32, tag="rec", name="rec")
                    nc.vector.reciprocal(rec, tr[:, 64:65])
                    otile = fin_pool.tile([128, 64], F32, tag="otile", name="otile")
                    nc.vector.tensor_scalar_mul(otile, tr[:, 0:64], rec)
                    nc.sync.dma_start(
                        out[ib * 128:(ib + 1) * 128, hd * 64:(hd + 1) * 64], otile
                    )

                # -- main loop: 8 passes = (head, i-half), ACT-paced --
                for p in range(2 * HPC):
                    h, ih = p // 2, p % 2
                    hp, ho = h // 2, (h % 2) * 64
                    ot = ot_ps.tile([65, 1024], F32, tag="ot", name="ot")
                    sts = [None] * NT
                    pts = [None] * NT

                    def emit_s(j):
                        st = st_ps.tile([128, 1024], F32, tag="st", name="st")
                        sts[j] = st
                        for s2 in range(2):
                            icol = ih * 1024 + s2 * 512
                            nc.tensor.matmul(
                                st[:, s2 * 512:(s2 + 1) * 512],
                                lhsT=kt_sb[hp][ho:ho + 64, j * 128:(j + 1) * 128],
                                rhs=qt_sb[hp][ho:ho + 64, icol:icol + 512],
                                start=True,
                                stop=True,
                            )

                    def emit_av(j):
                        for s2 in range(2):
                            nc.tensor.matmul(
                                ot[:, s2 * 512:(s2 + 1) * 512],
                                lhsT=v_sb[j][:, h * 65:(h + 1) * 65],
                                rhs=pts[j][:, s2 * 512:(s2 + 1) * 512],
                                start=(j == 0),
                                stop=(j == NT - 1),
                            )

                    emit_s(0)
                    emit_s(1)
                    for j in range(NT):
                        pt = pt_pool.tile([128, 1024], BF16, tag="pt", name="pt")
                        pts[j] = pt
                        nc.scalar.activation(
                            pt, sts[j], mybir.ActivationFunctionType.Exp, scale=SCALE
                        )
                        if j + 2 < NT:
                            emit_s(j + 2)
                        # batch AV matmuls in quads (two j's) so the PE can
                        # issue them back-to-back (pipelined, not isolated)
                        if j % 2 == 1:
                            emit_av(j - 1)
                            emit_av(j)
                        if p < 2:
                            # 1 projection group per iteration (V then QK m=1)
                            emit_filler()
                        elif j % 2 == 0:
                            # heads 0..2: output blocks as filler; 1 per 2 iters
                            hd = p // 2 - 1
                            emit_out_block(hd, (ih * NT + j) // 2)
                    # evacuate this pass's OT half
                    nc.vector.tensor_copy(
                        ots_sb[h][:, ih * 1024:(ih + 1) * 1024], ot
                    )

            # ---- tail: head 3's output blocks ----
            with tc.tile_pool(name="tr_ps", bufs=4, space="PSUM") as tr_ps:
                for ib in range(NT):
                    tr = tr_ps.tile([128, 65], F32, tag="tr", name="tr")
                    nc.tensor.transpose(
                        tr, ots_sb[3][:, ib * 128:(ib + 1) * 128], ident
                    )
                    rec = fin_pool.tile([128, 1], F32, tag="rec", name="rec")
                    nc.vector.reciprocal(rec, tr[:, 64:65])
                    otile = fin_pool.tile([128, 64], F32, tag="otile", name="otile")
                    nc.vector.tensor_scalar_mul(otile, tr[:, 0:64], rec)
                    nc.sync.dma_start(
                        out[ib * 128:(ib + 1) * 128, 192:256], otile
                    )

    nc.compile()
    return nc


def _get_nc():
    if "nc" not in _cache:
        _cache["nc"] = _build()
    return _cache["nc"]


def _shard_inputs(q, k, v, Wq, Wk, Wv, bq, bk, bv):
    in_maps = []
    for c in range(8):
        b, g = c // 2, c % 2
        sl = slice(g * EC, (g + 1) * EC)
        in_maps.append({
            "xq": np.ascontiguousarray(np.asarray(q)[b].T).astype(NP_BF16),
            "xk": np.ascontiguousarray(np.asarray(k)[b].T).astype(NP_BF16),
            "xv": np.ascontiguousarray(np.asarray(v)[b].T).astype(NP_BF16),
            "wq": np.ascontiguousarray(np.asarray(Wq)[:, sl]).astype(NP_BF16),
            "wk": np.ascontiguousarray(np.asarray(Wk)[:, sl]).astype(NP_BF16),
            "wv": np.ascontiguousarray(np.asarray(Wv)[:, sl]).astype(NP_BF16),
            "bqc": np.asarray(bq)[sl].reshape(EC, 1).astype(np.float32),
            "bkc": np.asarray(bk)[sl].reshape(EC, 1).astype(np.float32),
            "bvr": np.ascontiguousarray(
                np.broadcast_to(np.asarray(bv)[sl], (128, EC))
            ).astype(np.float32),
        })
    return in_maps


def kernel(q, k, v, Wq, Wk, Wv, bq, bk, bv, _trace=False):
    nc = _get_nc()
    in_maps = _shard_inputs(q, k, v, Wq, Wk, Wv, bq, bk, bv)
    res = run_bass_kernel_spmd(
        nc, in_maps, core_ids=list(range(8)), trace=_trace
    )
    out = np.empty((B, N, E), np.float32)
    for c in range(8):
        b, g = c // 2, c % 2
        out[b, :, g * EC:(g + 1) * EC] = res.results[c]["out"]
    if _trace:
        _cache["last_exec_time_ns"] = res.exec_time_ns
    return out
